# revision 10
# baseline (speedup 1.0000x reference)
"""BP-GNN message passing kernel for 8 Trainium2 NeuronCores.

Takes FULL inputs (as in reference.setup_inputs()), shards internally across
8 cores (edge parallelism by dst-range), runs 5 BP steps on device via Bass,
returns the FULL [N, C] log-belief output.

Math note: logH = -3*(1-I) collapses the per-edge logsumexp:
  raw[e,c] = log((1-a)*exp(t[e,c]) + a*S(e)),  a = e^-3, S = sum_c exp(t)
  log_msg  = log((1-a)*exp(t) + a*S) - log(S) - log(1+7a)
which needs only 8 exps + 2 logs per edge (vs 64 exps in the naive form).
No max-subtraction needed: t = log_b[src] - log_msg[rv] has t <= 3.3 and
max_c t >= -5.4, so exp/sum stay well inside f32 range.
"""
import sys
for _p in ('/opt/trn_rl_repo', '/root/.axon_site/_ro/trn_rl_repo'):
    if _p not in sys.path:
        import os
        if os.path.isdir(_p):
            sys.path.insert(0, _p)

import math
import numpy as np

from concourse import bass, mybir
from concourse.masks import make_identity

F32 = mybir.dt.float32
I32 = mybir.dt.int32

NCORES = 8
BLK = 16            # block size for two-level segmented reduction
CH = 16384          # rows per chunk (= 128 partitions x 128 cols)
A = math.exp(-3.0)
LN1MA = math.log(1.0 - A)          # ln(1-a)
SCL_LS = (1.0 + 7.0 * A) / (1.0 - A)  # fold log(1+7a)-log(1-a) into ln(scale*S)
A_OVER = A / (1.0 - A)


def _roundup(x, m):
    return (x + m - 1) // m * m


def preprocess(src, dst, rv, N, C, steps, ch1=16384, ch2=16384):
    """Host-side index preprocessing. Returns (consts dict, per-core input arrays)."""
    E = src.shape[0]
    src = src.astype(np.int64)
    dst = dst.astype(np.int64)
    rv = rv.astype(np.int64)
    NPC = N // NCORES
    LB = _roundup(NPC + 1, 128)          # padded log_b slice rows (128*51 = 6528 for N=50000)
    K51 = LB // 128

    q_of_edge = dst // NPC               # computing/aggregating core per edge
    powner = q_of_edge[rv]               # core that produces M-row for each edge

    # --- sigma order with per-(q,p,dst) runs padded to EVEN length ---
    # Each (powner, dst) run sits in consecutive sigma slots, padded to a
    # multiple of 2, so aggregation is a BLK=2 segmented reduce directly on
    # the P1 output (no pi-order re-gather of msig).
    order = np.lexsort((rv, dst, powner, q_of_edge))  # stable multi-key sort
    eq = q_of_edge[order]
    ep = powner[order]
    ed = dst[order]
    # run key per (q, p, dst); runs are consecutive in `order`
    runkey = (eq * NCORES + ep) * N + ed
    newrun = np.ones(E, dtype=bool)
    newrun[1:] = runkey[1:] != runkey[:-1]
    run_id = np.cumsum(newrun) - 1                 # per-edge run index
    nruns = int(run_id[-1]) + 1
    run_len = np.bincount(run_id, minlength=nruns)
    run_pad = ((run_len + 1) // 2) * 2
    run_q = eq[newrun]
    run_p = ep[newrun]
    run_node = ed[newrun] - run_q * NPC            # local dst node
    # padded start of each run within its (q,p) block
    blockkey = run_q * NCORES + run_p
    bstart = np.zeros(NCORES * NCORES + 1, dtype=np.int64)
    np.add.at(bstart, blockkey + 1, run_pad)
    blk_tot = np.add.reduceat(
        np.concatenate((bstart[1:], np.zeros(0, dtype=np.int64))),
        np.arange(NCORES * NCORES)) if False else bstart[1:].copy()
    # run start offset within block = exclusive cumsum of run_pad within block
    run_start = np.zeros(nruns, dtype=np.int64)
    cum = np.cumsum(run_pad)
    blk_first = np.zeros(nruns, dtype=bool)
    blk_first[0] = True
    blk_first[1:] = blockkey[1:] != blockkey[:-1]
    blk_base = np.zeros(nruns, dtype=np.int64)
    blk_base[blk_first] = np.concatenate(([0], cum[np.where(blk_first)[0][1:] - 1]))
    blk_base = np.maximum.accumulate(blk_base)
    run_start = np.concatenate(([0], cum[:-1])) - blk_base
    B = int(_roundup(int(blk_tot.max()), 256))
    EP = NCORES * B

    # slot of each real edge: block base + run_start + within-run rank
    within_run = np.arange(E, dtype=np.int64) - np.concatenate(
        ([0], np.cumsum(run_len)[:-1]))[run_id]
    slot_sorted = ep * B + run_start[run_id] + within_run
    slot_of_edge = np.full(E, -1, dtype=np.int64)
    slot_of_edge[order] = slot_sorted

    # per-core mask (1 for real slot), and 2-block -> bsums row tables
    MAXB = 1
    bs_info = []
    for q in range(NCORES):
        rsel = np.where(run_q == q)[0]
        msk = np.zeros(EP, dtype=np.float32)
        st = run_p[rsel] * B + run_start[rsel]
        # mark real slots
        for_len = run_len[rsel]
        flat = np.repeat(st, for_len) + (
            np.arange(int(for_len.sum()), dtype=np.int64)
            - np.repeat(np.concatenate(([0], np.cumsum(for_len)[:-1])), for_len))
        msk[flat] = 1.0
        # 2-blocks of each run -> (node, j) rows; j counts per node in
        # (dst-major) order for compactness
        nb_run = run_pad[rsel] // 2
        ro = np.argsort(run_node[rsel], kind='stable')
        nb_o = nb_run[ro]
        node_o = run_node[rsel][ro]
        st_o = st[ro]
        U = int(nb_o.sum())
        jrun = np.arange(U, dtype=np.int64) - np.repeat(
            np.concatenate(([0], np.cumsum(nb_o)[:-1])), nb_o)
        blk_slot = (np.repeat(st_o, nb_o) + 2 * jrun) // 2   # 2-block index
        cnt_node = np.bincount(np.repeat(node_o, nb_o), minlength=NPC)
        MAXB = max(MAXB, int(cnt_node.max()))
        jnode = np.arange(U, dtype=np.int64) - np.repeat(
            np.concatenate(([0], np.cumsum(cnt_node)))[:-1][np.repeat(node_o, nb_o)], 1)
        bs_info.append((msk, blk_slot, np.repeat(node_o, nb_o), jnode))


    # --- chunk layouts ---
    # P1 chunks: per sigma-block p, sub-chunks of <= CH slots (multiples of 128)
    p1_chunks = []      # (pblk, lo, cs)
    for p in range(NCORES):
        lo = 0
        while lo < B:
            cs = min(ch1, B - lo)
            p1_chunks.append((p, lo, cs))
            lo += cs
    TC1 = EP // 128     # total idx cols for per-slot arrays

    # helper: map a list of per-slot values (sigma slot id s) into the
    # [128, TC1] column layout: chunk-major, within chunk (p, k) p-major.
    def to_cols(vals, nslots, chunks):
        # vals: int array [nslots]; chunks: list of (base_slot, cs) in slot space
        arr = np.zeros((128, nslots // 128), dtype=np.int32)
        co = 0
        for base, cs in chunks:
            kc = cs // 128
            v = vals[base:base + cs].reshape(128, kc)
            arr[:, co:co + kc] = v
            co += kc
        return arr

    p1_slotchunks = [(p * B + lo, cs) for (p, lo, cs) in p1_chunks]

    # P3 chunks: per dest q, sub-chunks of B
    p3_chunks = []
    for q in range(NCORES):
        lo = 0
        while lo < B:
            cs = min(ch1, B - lo)
            p3_chunks.append((q, lo, cs))
            lo += cs
    p3_slotchunks = [(q * B + lo, cs) for (q, lo, cs) in p3_chunks]

    NPC_PAD = LB                      # blocksums node rows (incl trash at LB-1)
    TRASH = LB - 1

    BT = B + LB

    MAXB = _roundup(MAXB, 2)
    consts = dict(N=N, C=C, E=E, NPC=NPC, LB=LB, K51=K51, B=B, EP=EP,
                  MAXB=MAXB, BT=BT, TC1=TC1, steps=steps, CH1=ch1, CH2=ch2,
                  p1_chunks=p1_chunks, p3_chunks=p3_chunks)

    # --- per-core arrays ---
    per_core = []
    for q in range(NCORES):
        eids = np.where(q_of_edge == q)[0]
        slots = slot_of_edge[eids]                 # in [0, EP)
        # per-slot arrays (default pads)
        idxA = np.zeros(EP, dtype=np.int64)        # a2a_out row of log_b[src]
        idxProd = np.zeros(EP, dtype=np.int64)     # send-slot -> sigma slot of requested msg (on THIS core)
        r_src = src[eids] // NPC
        idxA[slots] = r_src * BT + B + (src[eids] - r_src * NPC)

        # requests: for each slot f of ANY core with powner(f)==q, the request
        # lands at send-slot (dest_core_of_f, position = slot_of_edge[f] % B)
        # and reads sigma slot of edge rv[f] (which lives on q).
        f_ids = np.where(powner == q)[0]           # edges whose M-row q produces
        dest = q_of_edge[f_ids]
        pos = slot_of_edge[f_ids] - dest * 0       # slot within f's core space
        # f's slot = powner(f)*B + within = q*B + within -> position within block = slot - q*B
        pos_in_block = slot_of_edge[f_ids] - q * B
        sendslot = dest * B + pos_in_block
        idxProd[sendslot] = slot_of_edge[rv[f_ids]] - 0  # sigma slot on q... see below

        # NOTE: slot_of_edge[rv[f]] is the slot of edge rv[f] *within its own
        # core's space* — and its core IS q (powner(f)=q). sigma slots on q
        # live in [0, EP) with block index = powner(rv[f]). Correct as-is.

        # mask (1=real slot) and 2-block -> bsums row table
        msk, blk_slot, blk_node, blk_j = bs_info[q]
        idxBS = np.full(EP // 2, TRASH * MAXB, dtype=np.int64)
        idxBS[blk_slot] = blk_node * MAXB + blk_j

        # column layouts (chunk-major, within chunk (p, k) p-major)
        idxA_c = to_cols(idxA, EP, p1_slotchunks)
        idxProd_c = to_cols(idxProd, EP, p3_slotchunks)
        # mask in the same per-slot column layout as idxA
        mask_c = np.zeros((128, TC1), dtype=np.float32)
        co = 0
        for base, cs in p1_slotchunks:
            kc = cs // 128
            mask_c[:, co:co + kc] = msk[base:base + cs].reshape(128, kc)
            co += kc
        # idxBS: per chunk, 2-block (p, kb) p-major
        idxBS_c = np.zeros((128, TC1 // 2), dtype=np.int32)
        co = 0
        for base, cs in p1_slotchunks:
            kb = cs // 256
            idxBS_c[:, co:co + kb] = idxBS[base // 2:base // 2 + cs // 2]\
                .reshape(128, kb)
            co += kb

        per_core.append(dict(idxA=idxA_c, idxProd=idxProd_c, mask=mask_c,
                             idxBS=idxBS_c))
    return consts, per_core


# ---------------------------------------------------------------------------
# Kernel builder
# ---------------------------------------------------------------------------

class P:
    """Op-record program: logical order with resolved semaphore waits."""
    def __init__(self):
        self.records = []          # (eng, waits[(sem,val)], emit_fn, incs[(sem,k)])
        self.cnt = {}

    def n(self, sem):
        return self.cnt.get(sem, 0)

    def op(self, eng, emit, incs=(), waits=()):
        waits = [(s, v) for (s, v) in waits if v > 0]
        self.records.append((eng, waits, emit, list(incs)))
        after = {}
        for s, k in incs:
            self.cnt[s] = self.cnt.get(s, 0) + k
            after[s] = self.cnt[s]
        return after

def build_kernel(consts, repeat=1):
    NPC, LB, K51 = consts['NPC'], consts['LB'], consts['K51']
    B, EP, MAXB, BT, TC1 = (consts['B'], consts['EP'],
                            consts['MAXB'], consts['BT'], consts['TC1'])
    C = consts['C']
    steps = consts['steps']
    p1_chunks, p3_chunks = consts['p1_chunks'], consts['p3_chunks']
    DIN = consts.get('DIN', 256)
    CH1 = consts.get('CH1', 16384)
    CH2 = consts.get('CH2', 16384)
    NX = LB
    KT = DIN // 128

    nc = bass.Bass(num_devices=NCORES)

    x_pad = nc.declare_dram_parameter('x_pad', [NX, DIN], F32, isOutput=False)
    w_in = nc.declare_dram_parameter('w_in', [DIN, C], F32, isOutput=False)
    b_in = nc.declare_dram_parameter('b_in', [128, K51 * C], F32, isOutput=False)
    idxA_in = nc.declare_dram_parameter('idxA', [128, TC1], I32, isOutput=False)
    idxProd_in = nc.declare_dram_parameter('idxProd', [128, TC1], I32, isOutput=False)
    mask_in = nc.declare_dram_parameter('mask', [128, TC1], F32, isOutput=False)
    idxBS_in = nc.declare_dram_parameter('idxBS', [128, TC1 // 2], I32, isOutput=False)
    out_t = nc.declare_dram_parameter('out', [LB, C], F32, isOutput=True)

    a2a_in = nc.dram_tensor('a2a_in', [NCORES * BT, C], F32)
    a2a_out = nc.dram_tensor('a2a_out', [NCORES * BT, C], F32)
    msig = nc.dram_tensor('msig', [EP + 128, C], F32)
    bsums = nc.dram_tensor('bsums', [LB * MAXB, C], F32)

    prog = P()
    ACT, DVE, GPS, SYN, PE = 'scalar', 'vector', 'gpsimd', 'sync', 'tensor'

    SEM_NAMES = ['sV', 'sA', 'sP', 'cc',
                 'mA0', 'mA1', 'mB0', 'mB1', 'mD0', 'mD1',
                 'mR0', 'mR1', 'mMini0', 'mMini1', 'mG0', 'mG1', 'mGS0', 'mGS1',
                 'mX0', 'mX1', 'mI', 'mZ', 'mP0', 'mT', 'mO']

    from contextlib import ExitStack
    with ExitStack() as ctx:
        block = ctx.enter_context(nc.Block())
        sems = {n: ctx.enter_context(nc.semaphore(n)) for n in SEM_NAMES}

        def sb(name, shape, dt=F32):
            return ctx.enter_context(nc.sbuf_tensor(name, shape, dt))

        idxA_sb = sb('idxA_sb', [128, TC1], I32)
        idxProd_sb = sb('idxProd_sb', [128, TC1], I32)
        mask_sb = sb('mask_sb', [128, TC1], F32)
        idxBS_sb = sb('idxBS_sb', [128, TC1 // 2], I32)
        Ab0 = sb('Ab0', [128, 1024]); Ab1 = sb('Ab1', [128, 1024])
        Bb0 = sb('Bb0', [128, 1024]); Bb1 = sb('Bb1', [128, 1024])
        Cb0 = sb('Cb0', [128, 1024]); Cb1 = sb('Cb1', [128, 1024])
        Db0 = sb('Db0', [128, 1024]); Db1 = sb('Db1', [128, 1024])
        Sb0 = sb('Sb0', [128, 128]); Sb1 = sb('Sb1', [128, 128])
        Lb0 = sb('Lb0', [128, 128]); Lb1 = sb('Lb1', [128, 128])
        BS0 = sb('BS0', [128, (CH1 // 256) * C]); BS1 = sb('BS1', [128, (CH1 // 256) * C])
        Gb0 = sb('Gb0', [128, 1024]); Gb1 = sb('Gb1', [128, 1024])
        BSsb = sb('BSsb', [128, K51 * MAXB * C])
        AGsb = sb('AGsb', [128, K51 * C])
        U2sb = sb('U2sb', [128, K51 * C])
        MXsb = sb('MXsb', [128, K51])
        S3sb = sb('S3sb', [128, K51])
        logb0_sb = sb('logb0_sb', [128, K51 * C])
        logb_sb = sb('logb_sb', [128, K51 * C])
        xc0 = sb('xc0', [128, 256]); xc1 = sb('xc1', [128, 256])
        xT_sb = sb('xT_sb', [128, 128])
        W_sb = sb('W_sb', [128, KT * C])
        bB_sb = sb('bB_sb', [128, K51 * C])
        ident = sb('ident', [128, 128])
        mlog8_sb = sb('mlog8_sb', [128, 1024])
        zero_sb = sb('zero_sb', [128, 1024])
        biasE = sb('biasE', [128, 1])
        psT = ctx.enter_context(nc.psum_tensor([128, 128], F32))
        psL = ctx.enter_context(nc.psum_tensor([128, C], F32))

        AB = [Ab0, Ab1]; BB = [Bb0, Bb1]; CB = [Cb0, Cb1]; DB = [Db0, Db1]
        SB = [Sb0, Sb1]; LB_ = [Lb0, Lb1]; BSB = [BS0, BS1]
        GB = [Gb0, Gb1]; XCH = [xc0, xc1]

        pr = prog

        # ================= INIT =================
        pr.op(SYN, lambda E: E.dma_start(out=idxA_sb[:], in_=idxA_in[:]), [('mI', 16)])
        pr.op(SYN, lambda E: E.dma_start(out=idxProd_sb[:], in_=idxProd_in[:]), [('mI', 16)])
        pr.op(SYN, lambda E: E.dma_start(out=mask_sb[:], in_=mask_in[:]), [('mI', 16)])
        pr.op(SYN, lambda E: E.dma_start(out=idxBS_sb[:], in_=idxBS_in[:]), [('mI', 16)])
        pr.op(SYN, lambda E: E.dma_start(
            out=W_sb[:].rearrange('p (t c) -> p t c', t=KT),
            in_=w_in[:].rearrange('(t p) c -> p t c', p=128)), [('mI', 16)])
        pr.op(SYN, lambda E: E.dma_start(out=bB_sb[:], in_=b_in[:]), [('mI', 16)])
        mI_tot = pr.n('mI')

        pr.op(GPS, lambda E: E.memset(mlog8_sb[:], -math.log(C)), [('sP', 1)])
        pr.op(GPS, lambda E: E.memset(zero_sb[:], 0.0), [('sP', 1)])
        pr.op(GPS, lambda E: E.memset(biasE[:], LN1MA), [('sP', 1)])
        pr.op(GPS, lambda E: E.memset(ident[:], 0.0), [('sP', 1)])
        pr.op(GPS, lambda E: E.affine_select(
            out=ident[:], in_=ident[:],
            compare_op=mybir.AluOpType.not_equal, fill=1.0, base=0,
            pattern=[[-1, 128]], channel_multiplier=1), [('sP', 1)])
        const_sp = pr.n('sP')

        # zero msig pad rows + bsums table (once per launch)
        pr.op(SYN, lambda E: E.dma_start(out=msig[EP:EP + 128, :], in_=zero_sb[:, :C]),
              [('mZ', 16)], waits=[('sP', 2)])
        nz = LB * MAXB
        lo = 0
        while lo < nz:
            csz = min(CH, nz - lo)
            csz = csz // 128 * 128
            if csz == 0:
                break
            pr.op(SYN, lambda E, lo=lo, csz=csz: E.dma_start(
                out=bsums[lo:lo + csz, :].rearrange('(p k) d -> p (k d)', p=128),
                in_=zero_sb[:, :csz // 128 * C]),
                [('mZ', 16)], waits=[('sP', 2)])
            lo += csz
        mZ_tot = pr.n('mZ')

        # logb0 = logsoftmax(x @ W + b)
        for t in range(K51):
            j = t % 2
            wX = [('mX%d' % j, 0)]
            if t >= 2:
                wX = [('sP', pr.n('sP'))]      # all PE work through t-1 done
            pr.op(SYN, lambda E, t=t, j=j: E.dma_start(
                out=XCH[j][:], in_=x_pad[t * 128:(t + 1) * 128, :]),
                [('mX%d' % j, 16)], waits=[(s, v) for (s, v) in wX if v > 0])
            xthr = pr.n('mX%d' % j)
            for h in range(KT):
                wt = [('mX%d' % j, xthr), ('sV', pr.n('sV'))]
                if t == 0 and h == 0:
                    wt.append(('sP', const_sp))
                pr.op(PE, lambda E, j=j, h=h: E.transpose(
                    out=psT[:], in_=XCH[j][:, h * 128:(h + 1) * 128], identity=ident[:]),
                    [('sP', 1)], waits=wt)
                pr.op(DVE, lambda E: E.tensor_copy(out=xT_sb[:], in_=psT[:]),
                      [('sV', 1)], waits=[('sP', pr.n('sP'))])
                pr.op(PE, lambda E, h=h: E.matmul(
                    out=psL[:], lhsT=xT_sb[:], rhs=W_sb[:, h * C:(h + 1) * C],
                    start=(h == 0), stop=(h == KT - 1)),
                    [('sP', 1)], waits=[('sV', pr.n('sV'))])
            pr.op(DVE, lambda E, t=t: E.tensor_copy(
                out=logb0_sb[:, t * C:(t + 1) * C], in_=psL[:]),
                [('sV', 1)], waits=[('sP', pr.n('sP'))])
        pr.op(DVE, lambda E: E.tensor_add(out=logb0_sb[:], in0=logb0_sb[:], in1=bB_sb[:]),
              [('sV', 1)], waits=[('mI', mI_tot)])
        pr.op(ACT, lambda E: E.activation(
            out=U2sb[:], in_=logb0_sb[:], func=mybir.ActivationFunctionType.Exp),
            [('sA', 1)], waits=[('sV', pr.n('sV'))])
        pr.op(DVE, lambda E: E.tensor_reduce(
            out=S3sb[:], in_=U2sb[:].rearrange('p (k c) -> p k c', c=C),
            axis=mybir.AxisListType.X, op=mybir.AluOpType.add),
            [('sV', 1)], waits=[('sA', pr.n('sA'))])
        pr.op(ACT, lambda E: E.activation(
            out=S3sb[:], in_=S3sb[:], func=mybir.ActivationFunctionType.Ln),
            [('sA', 1)], waits=[('sV', pr.n('sV'))])
        pr.op(DVE, lambda E: E.tensor_sub(
            out=logb0_sb[:].rearrange('p (k c) -> p k c', c=C),
            in0=logb0_sb[:].rearrange('p (k c) -> p k c', c=C),
            in1=S3sb[:].to_broadcast([128, K51, C])),
            [('sV', 1)], waits=[('sA', pr.n('sA'))])
        logb0_sv = pr.n('sV')

        # ================= REPEAT =================
        for rep in range(repeat):
            # P0: fill a2a_in M-regions with -log C; logb0 slices into tails
            gA_tots = [pr.n('mA0'), pr.n('mA1'), pr.n('mB0'), pr.n('mB1')]
            for q in range(NCORES):
                lo = 0
                while lo < B:
                    csz = min(CH, B - lo)
                    pr.op(SYN, lambda E, q=q, lo=lo, csz=csz: E.dma_start(
                        out=a2a_in[q * BT + lo:q * BT + lo + csz, :]
                            .rearrange('(p k) d -> p (k d)', p=128),
                        in_=mlog8_sb[:, :csz // 128 * C]),
                        [('mP0', 16)],
                        waits=[('sP', 2),
                               ('mA0', gA_tots[0]), ('mA1', gA_tots[1]),
                               ('mB0', gA_tots[2]), ('mB1', gA_tots[3])])
                    lo += csz
                pr.op(SYN, lambda E, q=q: E.dma_start(
                    out=a2a_in[q * BT + B:q * BT + B + LB, :]
                        .rearrange('(k p) d -> p k d', p=128),
                    in_=logb0_sb[:].rearrange('p (k d) -> p k d', d=C)),
                    [('mP0', 16)], waits=[('sV', logb0_sv)])
            pr.op(GPS, lambda E: E.collective_compute(
                'AllToAll', mybir.AluOpType.bypass,
                replica_groups=[list(range(NCORES))],
                ins=[a2a_in[:]], outs=[a2a_out[:]]),
                [('cc', 1)],
                waits=[('mP0', pr.n('mP0')), ('mZ', mZ_tot), ('mI', mI_tot),
                       ('mGS0', pr.n('mGS0')), ('mGS1', pr.n('mGS1')),
                       ('mO', pr.n('mO'))])
            cc_ready = pr.n('cc')

            for step in range(1, steps + 1):
                # ---- P1 ----
                subdone = {}; lndone = {}; stordone = {}; minidone = {}
                pending_scat = []

                def flush_scat():
                    for (j_, co_, nkb_, bsred_, ci_) in pending_scat:
                        for kb in range(nkb_):
                            pr.op(GPS, lambda E, j=j_, co=co_, kb=kb:
                                  E.indirect_dma_start(
                                      out=bsums[:],
                                      out_offset=bass.IndirectOffsetOnAxis(
                                          ap=idxBS_sb[:, co // 2 + kb:co // 2 + kb + 1],
                                          axis=0),
                                      in_=BSB[j][:, kb * C:(kb + 1) * C],
                                      in_offset=None),
                                  [('mMini%d' % j_, 16)],
                                  waits=([('sV', bsred_)] if kb == 0 else []))
                        minidone[ci_] = pr.n('mMini%d' % j_)
                    pending_scat.clear()

                co = 0
                for ci, (pblk, lo, cs) in enumerate(p1_chunks):
                    j = ci % 2
                    kc = cs // 128
                    sA_ = 'mA%d' % j; sB_ = 'mB%d' % j; sD_ = 'mD%d' % j
                    wA = [('cc', cc_ready)]
                    wB = [('cc', cc_ready)]
                    if ci >= 2:
                        wA.append(('sV', subdone[ci - 2]))
                        wB.append(('sV', subdone[ci - 2]))
                    for kk in range(kc):
                        pr.op(GPS, lambda E, j=j, co=co, kk=kk: E.indirect_dma_start(
                            out=AB[j][:, kk * C:(kk + 1) * C], out_offset=None,
                            in_=a2a_out[:],
                            in_offset=bass.IndirectOffsetOnAxis(
                                ap=idxA_sb[:, co + kk:co + kk + 1], axis=0)),
                            [(sA_, 16)], waits=(wA if kk == 0 else []))
                    thrA = pr.n(sA_)
                    flush_scat()
                    pr.op(SYN, lambda E, j=j, pblk=pblk, lo=lo, cs=cs, kc=kc: E.dma_start(
                        out=BB[j][:, :kc * C],
                        in_=a2a_out[pblk * BT + lo:pblk * BT + lo + cs, :]
                            .rearrange('(p k) d -> p (k d)', p=128)),
                        [(sB_, 16)], waits=wB)
                    thrB = pr.n(sB_)
                    wC = [(sA_, thrA), (sB_, thrB)]
                    if ci >= 2:
                        wC.append(('sA', lndone[ci - 2]))
                    pr.op(DVE, lambda E, j=j, kc=kc: E.tensor_sub(
                        out=CB[j][:, :kc * C], in0=AB[j][:, :kc * C],
                        in1=BB[j][:, :kc * C]),
                        [('sV', 1)], waits=wC)
                    subdone[ci] = pr.n('sV')
                    we = [('sV', subdone[ci])]
                    if rep == 0 and step == 1 and ci == 0:
                        we.append(('sP', const_sp))
                    pr.op(ACT, lambda E, j=j, kc=kc: E.activation(
                        out=CB[j][:, :kc * C], in_=CB[j][:, :kc * C],
                        func=mybir.ActivationFunctionType.Exp, bias=biasE[:, :1]),
                        [('sA', 1)], waits=we)
                    pr.op(DVE, lambda E, j=j, kc=kc: E.tensor_reduce(
                        out=SB[j][:, :kc],
                        in_=CB[j][:, :kc * C].rearrange('p (k c) -> p k c', c=C),
                        axis=mybir.AxisListType.X, op=mybir.AluOpType.add),
                        [('sV', 1)], waits=[('sA', pr.n('sA'))])
                    pr.op(ACT, lambda E, j=j, kc=kc: E.activation(
                        out=LB_[j][:, :kc], in_=SB[j][:, :kc],
                        func=mybir.ActivationFunctionType.Ln, scale=SCL_LS),
                        [('sA', 1)], waits=[('sV', pr.n('sV'))])
                    pr.op(DVE, lambda E, j=j, kc=kc: E.tensor_scalar_mul(
                        out=SB[j][:, :kc], in0=SB[j][:, :kc], scalar1=A_OVER),
                        [('sV', 1)], waits=[('sA', pr.n('sA'))])
                    pr.op(DVE, lambda E, j=j, kc=kc: E.tensor_add(
                        out=CB[j][:, :kc * C].rearrange('p (k c) -> p k c', c=C),
                        in0=CB[j][:, :kc * C].rearrange('p (k c) -> p k c', c=C),
                        in1=SB[j][:, :kc].to_broadcast([128, kc, C])),
                        [('sV', 1)])
                    pr.op(ACT, lambda E, j=j, kc=kc: E.activation(
                        out=CB[j][:, :kc * C], in_=CB[j][:, :kc * C],
                        func=mybir.ActivationFunctionType.Ln),
                        [('sA', 1)], waits=[('sV', pr.n('sV'))])
                    lndone[ci] = pr.n('sA')
                    wD = [('sA', lndone[ci])]
                    if ci >= 2:
                        wD.append((sD_, stordone[ci - 2]))
                    pr.op(DVE, lambda E, j=j, kc=kc: E.tensor_sub(
                        out=DB[j][:, :kc * C].rearrange('p (k c) -> p k c', c=C),
                        in0=CB[j][:, :kc * C].rearrange('p (k c) -> p k c', c=C),
                        in1=LB_[j][:, :kc].to_broadcast([128, kc, C])),
                        [('sV', 1)], waits=wD)
                    # zero the pad slots so the BLK=2 segment sums are exact
                    pr.op(DVE, lambda E, j=j, kc=kc, co=co: E.tensor_mul(
                        out=DB[j][:, :kc * C].rearrange('p (k c) -> p k c', c=C),
                        in0=DB[j][:, :kc * C].rearrange('p (k c) -> p k c', c=C),
                        in1=mask_sb[:, co:co + kc].to_broadcast([128, kc, C])),
                        [('sV', 1)], waits=[('mI', mI_tot)])
                    dready = pr.n('sV')
                    base = pblk * B + lo
                    pr.op(SYN, lambda E, j=j, base=base, cs=cs, kc=kc: E.dma_start(
                        out=msig[base:base + cs, :].rearrange('(p k) d -> p (k d)', p=128),
                        in_=DB[j][:, :kc * C]),
                        [(sD_, 16)], waits=[('sV', dready)])
                    stordone[ci] = pr.n(sD_)
                    # 2-block segment sums -> scatter into bsums
                    nkb = kc // 2
                    wred = []
                    if ci >= 2:
                        wred.append(('mMini%d' % j, minidone[ci - 2]))
                    pr.op(DVE, lambda E, j=j, kc=kc, nkb=nkb: E.tensor_reduce(
                        out=BSB[j][:, :nkb * C].rearrange('p (kb c) -> p kb c', c=C),
                        in_=DB[j][:, :kc * C].rearrange(
                            'p (kb t c) -> p kb c t', t=2, c=C),
                        axis=mybir.AxisListType.X, op=mybir.AluOpType.add),
                        [('sV', 1)], waits=wred)
                    bsred = pr.n('sV')
                    pending_scat.append((j, co, nkb, bsred, ci))
                    co += kc
                flush_scat()
                p1_store_tots = [('mD0', pr.n('mD0')), ('mD1', pr.n('mD1'))]
                mini_tots = [('mMini0', pr.n('mMini0')), ('mMini1', pr.n('mMini1'))]
                p1_gather_tots = [('mA0', pr.n('mA0')), ('mA1', pr.n('mA1')),
                                  ('mB0', pr.n('mB0')), ('mB1', pr.n('mB1'))]

                # ---- P3 (not last step) ----
                if step < steps:
                    co3 = 0
                    g3done = {}
                    for ci, (q, lo, cs) in enumerate(p3_chunks):
                        j = ci % 2
                        sG_ = 'mG%d' % j; sGS_ = 'mGS%d' % j
                        kc = cs // 128
                        w = list(p1_store_tots)
                        if ci >= 2:
                            w.append((sGS_, g3done[ci - 2]))
                        for kk in range(kc):
                            pr.op(GPS, lambda E, j=j, co3=co3, kk=kk: E.indirect_dma_start(
                                out=GB[j][:, kk * C:(kk + 1) * C], out_offset=None,
                                in_=msig[:],
                                in_offset=bass.IndirectOffsetOnAxis(
                                    ap=idxProd_sb[:, co3 + kk:co3 + kk + 1], axis=0)),
                                [(sG_, 16)], waits=(w if kk == 0 else []))
                        thrG = pr.n(sG_)
                        pr.op(SYN, lambda E, j=j, q=q, lo=lo, cs=cs, kc=kc: E.dma_start(
                            out=a2a_in[q * BT + lo:q * BT + lo + cs, :]
                                .rearrange('(p k) d -> p (k d)', p=128),
                            in_=GB[j][:, :kc * C]),
                            [(sGS_, 16)], waits=[(sG_, thrG)])
                        g3done[ci] = pr.n(sGS_)
                        co3 += kc

                # ---- P4 ----
                pr.op(SYN, lambda E: E.dma_start(
                    out=BSsb[:].rearrange('p (k j d) -> p k j d', j=MAXB, d=C),
                    in_=bsums[:].rearrange('(k p j) d -> p k j d', p=128, j=MAXB)),
                    [('mT', 16)], waits=mini_tots)
                pr.op(DVE, lambda E: E.tensor_reduce(
                    out=AGsb[:].rearrange('p (k c) -> p k c', c=C),
                    in_=BSsb[:].rearrange('p (k j c) -> p k c j', k=K51, j=MAXB, c=C),
                    axis=mybir.AxisListType.X, op=mybir.AluOpType.add),
                    [('sV', 1)], waits=[('mT', pr.n('mT'))])
                pr.op(DVE, lambda E: E.tensor_add(
                    out=AGsb[:], in0=AGsb[:], in1=logb0_sb[:]),
                    [('sV', 1)])
                pr.op(DVE, lambda E: E.tensor_reduce(
                    out=MXsb[:], in_=AGsb[:].rearrange('p (k c) -> p k c', c=C),
                    axis=mybir.AxisListType.X, op=mybir.AluOpType.max),
                    [('sV', 1)])
                pr.op(DVE, lambda E: E.tensor_sub(
                    out=U2sb[:].rearrange('p (k c) -> p k c', c=C),
                    in0=AGsb[:].rearrange('p (k c) -> p k c', c=C),
                    in1=MXsb[:].to_broadcast([128, K51, C])),
                    [('sV', 1)])
                pr.op(ACT, lambda E: E.activation(
                    out=U2sb[:], in_=U2sb[:], func=mybir.ActivationFunctionType.Exp),
                    [('sA', 1)], waits=[('sV', pr.n('sV'))])
                pr.op(DVE, lambda E: E.tensor_reduce(
                    out=S3sb[:], in_=U2sb[:].rearrange('p (k c) -> p k c', c=C),
                    axis=mybir.AxisListType.X, op=mybir.AluOpType.add),
                    [('sV', 1)], waits=[('sA', pr.n('sA'))])
                pr.op(ACT, lambda E: E.activation(
                    out=S3sb[:], in_=S3sb[:], func=mybir.ActivationFunctionType.Ln),
                    [('sA', 1)], waits=[('sV', pr.n('sV'))])
                pr.op(DVE, lambda E: E.tensor_add(
                    out=S3sb[:], in0=S3sb[:], in1=MXsb[:]),
                    [('sV', 1)], waits=[('sA', pr.n('sA'))])
                pr.op(DVE, lambda E: E.tensor_sub(
                    out=logb_sb[:].rearrange('p (k c) -> p k c', c=C),
                    in0=AGsb[:].rearrange('p (k c) -> p k c', c=C),
                    in1=S3sb[:].to_broadcast([128, K51, C])),
                    [('sV', 1)])
                logb_sv = pr.n('sV')

                if step < steps:
                    for q in range(NCORES):
                        pr.op(SYN, lambda E, q=q: E.dma_start(
                            out=a2a_in[q * BT + B:q * BT + B + LB, :]
                                .rearrange('(k p) d -> p k d', p=128),
                            in_=logb_sb[:].rearrange('p (k d) -> p k d', d=C)),
                            [('mO', 16)], waits=[('sV', logb_sv)])
                    a2a_waits = [('mO', pr.n('mO')),
                                 ('mGS0', pr.n('mGS0')), ('mGS1', pr.n('mGS1'))]
                    a2a_waits += p1_gather_tots
                    pr.op(GPS, lambda E: E.collective_compute(
                        'AllToAll', mybir.AluOpType.bypass,
                        replica_groups=[list(range(NCORES))],
                        ins=[a2a_in[:]], outs=[a2a_out[:]]),
                        [('cc', 1)], waits=a2a_waits)
                    cc_ready = pr.n('cc')
                else:
                    pr.op(SYN, lambda E: E.dma_start(
                        out=out_t[:].rearrange('(k p) d -> p k d', p=128),
                        in_=logb_sb[:].rearrange('p (k d) -> p k d', d=C)),
                        [('mO', 16)], waits=[('sV', logb_sv)])

        # final totals for drains
        finals = {n: prog.n(n) for n in SEM_NAMES}

        serial_sem = {DVE: 'sV', ACT: 'sA', PE: 'sP', GPS: 'sP'}
        SYN_SEMS = ['mI', 'mZ', 'mP0', 'mB0', 'mB1', 'mD0', 'mD1',
                    'mGS0', 'mGS1', 'mX0', 'mX1', 'mT', 'mO']
        GPS_SEMS = ['mA0', 'mA1', 'mR0', 'mR1', 'mMini0', 'mMini1', 'mG0', 'mG1', 'cc']

        def emit_for(eng_name):
            def fn(E):
                cnt = {}
                last_serial = 0
                ssem = serial_sem.get(eng_name)
                for (eng, waits, emit, incs) in prog.records:
                    if eng != eng_name:
                        for s, k in incs:
                            cnt[s] = cnt.get(s, 0) + k
                        continue
                    wl = list(waits)
                    if ssem is not None and any(s == ssem for s, _ in incs):
                        if last_serial > 0:
                            wl.append((ssem, last_serial))
                    for s, v in wl:
                        E.wait_ge(sems[s], v)
                    inst = emit(E)
                    for s, k in incs:
                        inst.then_inc(sems[s], k)
                        cnt[s] = cnt.get(s, 0) + k
                        if s == ssem:
                            last_serial = cnt[s]
                if eng_name == SYN:
                    for s in SYN_SEMS:
                        if finals[s] > 0:
                            E.wait_ge(sems[s], finals[s])
                if eng_name == GPS:
                    for s in GPS_SEMS:
                        if finals[s] > 0:
                            E.wait_ge(sems[s], finals[s])
            return fn

        block.sync(emit_for(SYN))
        block.vector(emit_for(DVE))
        block.scalar(emit_for(ACT))
        block.gpsimd(emit_for(GPS))
        block.tensor(emit_for(PE))

    return nc

# ---------------------------------------------------------------------------
# Host entry
# ---------------------------------------------------------------------------

_CACHE = {}


def _build_inputs(x, W, b, consts, per_core):
    N, NPC, LB, K51 = consts['N'], consts['NPC'], consts['LB'], consts['K51']
    C = consts['C']
    in_maps = []
    for q in range(NCORES):
        xs = np.zeros((LB, x.shape[1]), np.float32)
        xs[:NPC] = x[q * NPC:(q + 1) * NPC]
        b_b = np.tile(b.astype(np.float32)[None, :], (128, K51))
        m = dict(x_pad=xs, w_in=W.astype(np.float32), b_in=b_b,
                 idxA=per_core[q]['idxA'], idxProd=per_core[q]['idxProd'],
                 mask=per_core[q]['mask'], idxBS=per_core[q]['idxBS'])
        in_maps.append(m)
    return in_maps


def kernel(x, W, b, edge_index, rv, repeat=1, use_sim=False, steps=5,
           ch1=16384, ch2=16384):
    x = np.asarray(x); W = np.asarray(W); b = np.asarray(b)
    edge_index = np.asarray(edge_index); rv = np.asarray(rv)
    N, C = x.shape[0], W.shape[1]
    src, dst = edge_index[0], edge_index[1]

    key = (N, C, src.shape[0], steps, repeat, ch1, ch2,
           hash(src.tobytes()) ^ hash(dst.tobytes()) ^ hash(rv.tobytes()))
    if key in _CACHE:
        consts, per_core, run = _CACHE[key]
    else:
        consts, per_core = preprocess(src, dst, rv, N, C, steps, ch1=ch1, ch2=ch2)
        consts['DIN'] = x.shape[1]
        nc = build_kernel(consts, repeat=repeat)
        if use_sim:
            run = ('sim', nc)
        else:
            from concourse.bass_utils import run_bass_kernel_spmd
            run = ('hw', nc)
        _CACHE[key] = (consts, per_core, run)

    in_maps = _build_inputs(x, W, b, consts, per_core)
    mode, nc = run
    NPC = consts['N'] // NCORES
    if mode == 'sim':
        from concourse.bass_interp import MultiCoreSim
        sim = MultiCoreSim(nc, num_cores=NCORES)
        for q in range(NCORES):
            for k, v in in_maps[q].items():
                sim.cores[q].tensor(k)[:] = v
        sim.simulate()
        outs = [np.array(sim.cores[q].tensor('out')) for q in range(NCORES)]
    else:
        from concourse.bass_utils import run_bass_kernel_spmd
        res = run_bass_kernel_spmd(nc, in_maps, list(range(NCORES)))
        outs = [res.results[q]['out'] for q in range(NCORES)]
    return np.concatenate([o[:NPC] for o in outs], axis=0)

